# revision 16
# baseline (speedup 1.0000x reference)
"""Trainium2 Bass kernel for CustomMultiheadAttention.

Problem: S=2048, B=2, E=768, H=12, D=64.
  Q = query @ W_Q.T + b_Q   (same for K, V)      [S, B, E]
  per head: scores = Qh @ Kh.T / sqrt(D)         [B*H, S, S]
  attn = softmax(scores, axis=-1)
  ctx = attn @ Vh                                [B*H, S, D]
  output = ctx-merge @ W_O.T + b_O               [S, B, E]
  returns (output, attn_weights)

Sharding: 24 (batch*head) pairs over 8 cores -> 3 heads of one batch
element per core.  Core c: batch b = c//4, global heads 3*(c%4)+i,
i.e. E-channels [192*(c%4), 192*(c%4)+192).

Device-side layout is fully "transposed": the projections produce
QT/KT/VT = [channels, seq] so that
  scoresT[k, q] = KT_slice.T @ QT_chunk          (PE contracts over d=64)
  exp (ACT, scale=1/8, no max-subtraction: |scores| <~ 6, safe in fp32)
  Z[q] = column sums via DVE tree-add + ones-matmul
  attnT = expT * (1/Z)  broadcast via gpsimd partition_broadcast
  ctxT[d, q] = sum_kb V[kb].T-block @ expT[kb]   (V natural via PE transpose)
  out_partial[s, e] = sum_h ctxT_h.T-slice @ WO_h
attn_weights is written in a device-friendly [h, qc, p, kb, q] layout and
untransposed on the host; partial outputs are summed on the host (+ b_O).

Matmuls run as float32r (full-rate on PE; plain fp32 is 4 cycles/row).
"""

import os
import sys

import numpy as np

for _p in ("/opt/trn_rl_repo", "/root/.axon_site/_ro/trn_rl_repo"):
    if os.path.isdir(_p) and _p not in sys.path:
        sys.path.insert(0, _p)

import concourse.bass as bass
import concourse.tile as tile
from concourse import bacc as _bacc
from concourse import mybir
from concourse import bass_utils as _bu
from concourse.bass_utils import run_bass_kernel_spmd

# Route IO DMAs through DGE so DMACopy instructions are not limited to the
# single sync-wait slot of the static PSEUDO_DMA_DIRECT2D lowering.
if not getattr(_bu, "_dge_io_patch", False):
    _orig_walrus_args = _bu.get_walrus_args

    def _walrus_args_dge(*a, **k):
        return [*_orig_walrus_args(*a, **k), "--dge-levels=io"]

    _bu.get_walrus_args = _walrus_args_dge
    _bu._dge_io_patch = True

S, B, E, H = 2048, 2, 768, 12
D = 64
NCORES = 8
HPC = 3                  # heads per core
C = HPC * D              # 192 channels per core
EB = E // 128            # 6 contraction blocks for projections
QC = 4                   # q-chunks
QCH = S // QC            # 512
KB = S // 128            # 16 k-blocks
F32 = mybir.dt.float32
F32R = mybir.dt.float32r

ActF = mybir.ActivationFunctionType

_CACHE = {}


def _build(use_f32r=True):
    nc = _bacc.Bacc(None, target_bir_lowering=False, debug=False)
    xq = nc.declare_dram_parameter("xq_t", [E, S], F32, isOutput=False)
    xk = nc.declare_dram_parameter("xk_t", [E, S], F32, isOutput=False)
    xv = nc.declare_dram_parameter("xv_t", [E, S], F32, isOutput=False)
    wq = nc.declare_dram_parameter("wq_t", [E, C], F32, isOutput=False)
    wk = nc.declare_dram_parameter("wk_t", [E, C], F32, isOutput=False)
    wv = nc.declare_dram_parameter("wv_t", [E, C], F32, isOutput=False)
    wo = nc.declare_dram_parameter("wo_t", [C, E], F32, isOutput=False)
    bp = nc.declare_dram_parameter("b_pack", [D, 9], F32, isOutput=False)
    idin = nc.declare_dram_parameter("ident64", [D, D], F32, isOutput=False)
    attn_o = nc.declare_dram_parameter(
        "attn_t", [HPC, QC, 128, KB, QCH], F32, isOutput=True
    )
    out_o = nc.declare_dram_parameter("out_partial", [KB, 128, E], F32, isOutput=True)

    def r(ap):
        return ap.bitcast(F32R) if use_f32r else ap

    from contextlib import ExitStack

    with tile.TileContext(nc) as tc, ExitStack() as ctx:
        consts = ctx.enter_context(tc.tile_pool(name="consts", bufs=1))

        qkv_sb = {}
        vh = []
        qkvpool = ctx.enter_context(tc.tile_pool(name="qkv", bufs=1))

        # ---- Phase 1: const loads + projections (contract over E, 6 blocks) ----
        with (
            tc.tile_pool(name="wstg", bufs=2) as wstg,
            tc.tile_pool(name="xin", bufs=6) as xin_pool,
            tc.tile_pool(name="vt", bufs=1) as vt_pool,
            tc.tile_pool(name="pproj", bufs=4, space="PSUM") as pproj,
            tc.tile_pool(name="ptr", bufs=2, space="PSUM") as ptr,
        ):
            w_sb = {}
            for name, dram in (("q", wq), ("k", wk), ("v", wv)):
                td = wstg.tile([128, EB, C], F32, tag="wd")
                nc.gpsimd.dma_start(
                    out=td, in_=dram[:].rearrange("(eb p) c -> p eb c", p=128)
                )
                t = consts.tile([128, EB, C], F32, tag=f"w_{name}")
                nc.vector.tensor_copy(t[:].bitcast(F32R), td)
                w_sb[name] = t
            wo_sb = []
            for h in range(HPC):
                td = wstg.tile([D, E], F32, tag="wod")
                nc.gpsimd.dma_start(out=td, in_=wo[h * D : (h + 1) * D, :])
                t = consts.tile([D, E], F32, tag=f"wo_{h}")
                nc.vector.tensor_copy(t[:].bitcast(F32R), td)
                wo_sb.append(t)
            b_sb = consts.tile([D, 9], F32)
            nc.gpsimd.dma_start(out=b_sb, in_=bp[:])
            ones_sb = consts.tile([128, 1], F32)
            nc.vector.memset(ones_sb, 1.0)
            ones_row = consts.tile([1, 128], F32)
            nc.vector.memset(ones_row, 1.0)
            identd = wstg.tile([D, D], F32, tag="identd")
            nc.gpsimd.dma_start(out=identd, in_=idin[:])
            ident = consts.tile([D, D], F32)
            nc.vector.tensor_copy(ident, identd)
            # PE observer: one transpose consuming the last DVE funnel copy so
            # later matmuls see all funnel ticks without extra waits
            obs = ptr.tile([D, D], F32, tag="ptr")
            nc.tensor.transpose(obs, ident, ident)

            for ti, (tname, dram) in enumerate((("q", xq), ("k", xk), ("v", xv))):
                xin = []
                for eb in range(EB):
                    xt = xin_pool.tile([128, S], F32, tag="xin")
                    nc.gpsimd.dma_start(
                        out=xt[:].bitcast(F32R),
                        in_=dram[eb * 128 : (eb + 1) * 128, :].bitcast(F32R),
                    )
                    xin.append(xt)
                for h in range(HPC):
                    if tname == "v":
                        qt = vt_pool.tile([D, S], F32, tag=f"vt{h}")
                    else:
                        qt = qkvpool.tile([D, S], F32, tag=f"{tname}{h}")
                    for nch in range(QC):
                        ps = pproj.tile([D, QCH], F32, tag="pproj")
                        for eb in range(EB):
                            nc.tensor.matmul(
                                ps,
                                lhsT=r(w_sb[tname][:, eb, h * D : (h + 1) * D]),
                                rhs=r(xin[eb][:, nch * QCH : (nch + 1) * QCH]),
                                start=(eb == 0),
                                stop=(eb == EB - 1),
                            )
                        nc.vector.tensor_scalar_add(
                            out=qt[:, nch * QCH : (nch + 1) * QCH].bitcast(F32R),
                            in0=ps,
                            scalar1=b_sb[:, ti * HPC + h : ti * HPC + h + 1],
                        )
                    qkv_sb[(tname, h)] = qt
                if tname != "v":
                    # PE observer: absorb this tensor's last bias-add tick
                    obs = ptr.tile([128, D], F32, tag="ptr")
                    nc.tensor.transpose(
                        obs, qkv_sb[(tname, HPC - 1)][:, S - 128 : S], ident
                    )

            # V natural layout per head: [128 (k-block rows), kb, 64] via PE transpose
            for h in range(HPC):
                vt = qkv_sb[("v", h)]
                vht = qkvpool.tile([128, KB, D], F32, tag=f"vh{h}")
                for kb in range(KB):
                    pst = ptr.tile([128, D], F32, tag="ptr")
                    nc.tensor.transpose(
                        pst, vt[:, kb * 128 : (kb + 1) * 128], ident
                    )
                    nc.vector.tensor_copy(vht[:, kb, :].bitcast(F32R), pst)
                vh.append(vht)

        # ---- Phase 2: attention + output projection ----
        with (
            tc.tile_pool(name="psc", bufs=2, space="PSUM") as psc,
            tc.tile_pool(name="psz", bufs=1, space="PSUM") as psz,
            tc.tile_pool(name="psrb", bufs=1, space="PSUM") as psrb,
            tc.tile_pool(name="psctx", bufs=2, space="PSUM") as psctx,
            tc.tile_pool(name="pso", bufs=1, space="PSUM") as pso_pool,
            tc.tile_pool(name="expp", bufs=2) as expp,
            tc.tile_pool(name="zp", bufs=1) as zp,
            tc.tile_pool(name="rp", bufs=2) as rp,
            tc.tile_pool(name="rbp", bufs=1) as rbp,
            tc.tile_pool(name="ctxup", bufs=2) as ctxup,
            tc.tile_pool(name="ctxtp", bufs=6) as ctxtp,
            tc.tile_pool(name="outp", bufs=2) as outp,
            tc.tile_pool(name="stgp", bufs=2) as stgp,
        ):
            for qc in range(QC):
                ctxt_q = []
                for h in range(HPC):
                    qt = qkv_sb[("q", h)]
                    kt = qkv_sb[("k", h)]
                    exp_chunk = expp.tile([128, KB, QCH], F32, tag="exp")
                    zsum = zp.tile([128, QCH], F32, tag="zsum")
                    for kb in range(KB):
                        ps = psc.tile([128, QCH], F32, tag="psc")
                        nc.tensor.matmul(
                            ps,
                            lhsT=r(kt[:, kb * 128 : (kb + 1) * 128]),
                            rhs=r(qt[:, qc * QCH : (qc + 1) * QCH]),
                            start=True,
                            stop=True,
                        )
                        nc.scalar.activation(
                            out=exp_chunk[:, kb, :].bitcast(F32R),
                            in_=ps,
                            func=ActF.Exp,
                            scale=0.125,
                        )
                        if kb == 1:
                            nc.vector.tensor_add(
                                zsum, exp_chunk[:, 0, :], exp_chunk[:, 1, :]
                            )
                        elif kb > 1:
                            nc.vector.tensor_add(zsum, zsum, exp_chunk[:, kb, :])
                    # Z row: reduce zsum over partitions with a ones-matmul
                    # (plain fp32: exact and cheap at 1 matmul per chunk)
                    zrow_ps = psz.tile([1, QCH], F32, tag="zrow")
                    nc.tensor.matmul(zrow_ps, lhsT=ones_sb, rhs=zsum, start=True, stop=True)
                    recip = rp.tile([1, QCH], F32, tag="recip")
                    nc.vector.reciprocal(recip, zrow_ps)
                    # broadcast 1/Z to all partitions: ones-column (K=1) matmul
                    rb_ps = psrb.tile([128, QCH], F32, tag="rbps")
                    nc.tensor.matmul(rb_ps, lhsT=ones_row, rhs=recip, start=True, stop=True)
                    recipB = rbp.tile([128, QCH], F32, tag="recipB")
                    nc.vector.tensor_copy(recipB, rb_ps)

                    # ctx (on unnormalized exp), then scale columns by 1/Z
                    cps = psctx.tile([D, QCH], F32, tag="cps")
                    for kb in range(KB):
                        nc.tensor.matmul(
                            cps,
                            lhsT=r(vh[h][:, kb, :]),
                            rhs=r(exp_chunk[:, kb, :]),
                            start=(kb == 0),
                            stop=(kb == KB - 1),
                        )
                    ctxu = ctxup.tile([D, QCH], F32, tag="ctxu")
                    nc.scalar.activation(out=ctxu, in_=cps, func=ActF.Copy)
                    ctxt = ctxtp.tile([D, QCH], F32, tag="ctxt")
                    nc.vector.tensor_mul(ctxt[:].bitcast(F32R), ctxu, recipB[0:D, :])
                    ctxt_q.append(ctxt)

                    # normalize attn through staging tiles (exp_chunk is
                    # f32r-consumed; attn output stays full fp32)
                    for g in range(4):
                        stg = stgp.tile([128, 4, QCH], F32, tag="stg")
                        nc.vector.tensor_mul(
                            stg,
                            exp_chunk[:, g * 4 : (g + 1) * 4, :],
                            recipB[:, None, :].broadcast_to([128, 4, QCH]),
                        )
                        nc.gpsimd.dma_start(
                            out=attn_o[h, qc, :, g * 4 : (g + 1) * 4, :], in_=stg
                        )

                # output projection for this q-chunk (s == q rows)
                for sb in range(4):
                    poa = pso_pool.tile([128, 512], F32, tag="poa")
                    pob = pso_pool.tile([128, 256], F32, tag="pob")
                    for h in range(HPC):
                        lh = r(ctxt_q[h][:, sb * 128 : (sb + 1) * 128])
                        nc.tensor.matmul(
                            poa, lhsT=lh, rhs=r(wo_sb[h][:, 0:512]),
                            start=(h == 0), stop=(h == HPC - 1),
                        )
                        nc.tensor.matmul(
                            pob, lhsT=lh, rhs=r(wo_sb[h][:, 512:768]),
                            start=(h == 0), stop=(h == HPC - 1),
                        )
                    ot = outp.tile([128, E], F32, tag="out")
                    nc.vector.tensor_copy(ot[:, 0:512], poa)
                    nc.vector.tensor_copy(ot[:, 512:768], pob)
                    nc.gpsimd.dma_start(out=out_o[qc * 4 + sb], in_=ot)
    nc.compile()
    return nc


def _get_nc():
    use_f32r = os.environ.get("BASS_MM_DTYPE", "f32r") == "f32r"
    key = ("nc", use_f32r)
    if key not in _CACHE:
        _CACHE[key] = _build(use_f32r)
    return _CACHE[key]


def make_in_maps(query, key, value, W_Q, W_K, W_V, b_Q, b_K, b_V, W_O, b_O):
    query = np.asarray(query, np.float32)
    key = np.asarray(key, np.float32)
    value = np.asarray(value, np.float32)
    in_maps = []
    for c in range(NCORES):
        b, hs = c // 4, c % 4
        ch0 = C * hs
        bpk = np.zeros((D, 9), np.float32)
        for ti, bb in enumerate((b_Q, b_K, b_V)):
            bb = np.asarray(bb, np.float32)
            for h in range(HPC):
                bpk[:, ti * HPC + h] = bb[ch0 + h * D : ch0 + (h + 1) * D]
        in_maps.append(
            {
                "xq_t": np.ascontiguousarray(query[:, b, :].T),
                "xk_t": np.ascontiguousarray(key[:, b, :].T),
                "xv_t": np.ascontiguousarray(value[:, b, :].T),
                "wq_t": np.ascontiguousarray(np.asarray(W_Q)[ch0 : ch0 + C, :].T),
                "wk_t": np.ascontiguousarray(np.asarray(W_K)[ch0 : ch0 + C, :].T),
                "wv_t": np.ascontiguousarray(np.asarray(W_V)[ch0 : ch0 + C, :].T),
                "wo_t": np.ascontiguousarray(np.asarray(W_O)[:, ch0 : ch0 + C].T),
                "b_pack": bpk,
                "ident64": np.eye(D, dtype=np.float32),
            }
        )
    return in_maps


def assemble(results, b_O):
    out = np.zeros((S, B, E), np.float32)
    attn = np.empty((B * H, S, S), np.float32)
    for c in range(NCORES):
        b, hs = c // 4, c % 4
        out[:, b, :] += results[c]["out_partial"].reshape(S, E)
        a = results[c]["attn_t"]  # [HPC, QC, 128, KB, QCH]
        n0 = H * b + HPC * hs
        attn[n0 : n0 + HPC] = a.transpose(0, 1, 4, 3, 2).reshape(HPC, S, S)
    out += np.asarray(b_O, np.float32)[None, None, :]
    return out, attn


_LAST = {}


def _ensure_ntff_hook():
    """bass_utils' trace path imports antenv.axon_hooks, which this image
    lacks. Recreate it: a tiny registry returning the ctypes NTFF hook
    from trn_agent_boot (dlopens libaxon_pjrt.so directly)."""
    import types

    if "antenv.axon_hooks" in sys.modules:
        return
    try:
        from trn_agent_boot.trn_boot import _ntff_profile_via_ctypes

        hook = _ntff_profile_via_ctypes("/opt/axon/libaxon_pjrt.so")
    except Exception:
        hook = None
    mod = types.ModuleType("antenv.axon_hooks")
    mod._hook = hook
    mod.get_axon_ntff_profile_hook = lambda: mod._hook
    mod.set_axon_ntff_profile_hook = lambda h: setattr(mod, "_hook", h)
    sys.modules["antenv.axon_hooks"] = mod


def kernel(query, key, value, W_Q, W_K, W_V, b_Q, b_K, b_V, W_O, b_O):
    nc = _get_nc()
    in_maps = make_in_maps(
        query, key, value, W_Q, W_K, W_V, b_Q, b_K, b_V, W_O, b_O
    )
    trace = os.environ.get("BASS_KERNEL_TRACE", "0") == "1"
    if trace:
        _ensure_ntff_hook()
    res = run_bass_kernel_spmd(nc, in_maps, core_ids=list(range(NCORES)), trace=trace)
    _LAST["exec_time_ns"] = res.exec_time_ns
    _LAST["profile_json"] = res.profile_json
    if res.exec_time_ns is not None:
        print(f"HW exec time: {res.exec_time_ns} ns")
    return assemble(res.results, b_O)


# revision 18
# speedup vs baseline: 1.2316x; 1.2316x over previous
"""Trainium2 Bass kernel for CustomMultiheadAttention.

Problem: S=2048, B=2, E=768, H=12, D=64.
  Q = query @ W_Q.T + b_Q   (same for K, V)      [S, B, E]
  per head: scores = Qh @ Kh.T / sqrt(D)         [B*H, S, S]
  attn = softmax(scores, axis=-1)
  ctx = attn @ Vh                                [B*H, S, D]
  output = ctx-merge @ W_O.T + b_O               [S, B, E]
  returns (output, attn_weights)

Sharding: 24 (batch*head) pairs over 8 cores -> 3 heads of one batch
element per core.  Core c: batch b = c//4, global heads 3*(c%4)+i,
i.e. E-channels [192*(c%4), 192*(c%4)+192).

Device-side layout is fully "transposed": the projections produce
QT/KT/VT = [channels, seq] so that
  scoresT[k, q] = KT_slice.T @ QT_chunk          (PE contracts over d=64)
  exp (ACT, scale=1/8, no max-subtraction: |scores| <~ 6, safe in fp32)
  Z[q] = column sums via DVE tree-add + ones-matmul
  attnT = expT * (1/Z)  broadcast via gpsimd partition_broadcast
  ctxT[d, q] = sum_kb V[kb].T-block @ expT[kb]   (V natural via PE transpose)
  out_partial[s, e] = sum_h ctxT_h.T-slice @ WO_h
attn_weights is written in a device-friendly [h, qc, p, kb, q] layout and
untransposed on the host; partial outputs are summed on the host (+ b_O).

Matmuls run as float32r (full-rate on PE; plain fp32 is 4 cycles/row).
"""

import os
import sys

import numpy as np

for _p in ("/opt/trn_rl_repo", "/root/.axon_site/_ro/trn_rl_repo"):
    if os.path.isdir(_p) and _p not in sys.path:
        sys.path.insert(0, _p)

import concourse.bass as bass
import concourse.tile as tile
from concourse import bacc as _bacc
from concourse import mybir
from concourse import bass_utils as _bu
from concourse.bass_utils import run_bass_kernel_spmd

# Route IO DMAs through DGE so DMACopy instructions are not limited to the
# single sync-wait slot of the static PSEUDO_DMA_DIRECT2D lowering.
if not getattr(_bu, "_dge_io_patch", False):
    _orig_walrus_args = _bu.get_walrus_args

    def _walrus_args_dge(*a, **k):
        return [*_orig_walrus_args(*a, **k), "--dge-levels=io"]

    _bu.get_walrus_args = _walrus_args_dge
    _bu._dge_io_patch = True

S, B, E, H = 2048, 2, 768, 12
D = 64
NCORES = 8
HPC = 3                  # heads per core
C = HPC * D              # 192 channels per core
EB = E // 128            # 6 contraction blocks for projections
QC = 4                   # q-chunks
QCH = S // QC            # 512
KB = S // 128            # 16 k-blocks
F32 = mybir.dt.float32
F32R = mybir.dt.float32r

ActF = mybir.ActivationFunctionType

_CACHE = {}


def _build(use_f32r=True):
    nc = _bacc.Bacc(None, target_bir_lowering=False, debug=False)
    xq = nc.declare_dram_parameter("xq_t", [E, S], F32, isOutput=False)
    xk = nc.declare_dram_parameter("xk_t", [E, S], F32, isOutput=False)
    xv = nc.declare_dram_parameter("xv_t", [E, S], F32, isOutput=False)
    wq = nc.declare_dram_parameter("wq_t", [E, C], F32, isOutput=False)
    wk = nc.declare_dram_parameter("wk_t", [E, C], F32, isOutput=False)
    wv = nc.declare_dram_parameter("wv_t", [E, C], F32, isOutput=False)
    wo = nc.declare_dram_parameter("wo_t", [C, E], F32, isOutput=False)
    bp = nc.declare_dram_parameter("b_pack", [D, 9], F32, isOutput=False)
    idin = nc.declare_dram_parameter("ident64", [D, D], F32, isOutput=False)
    attn_o = nc.declare_dram_parameter(
        "attn_t", [HPC, QC, 128, KB, QCH], F32, isOutput=True
    )
    out_o = nc.declare_dram_parameter("out_partial", [KB, 128, E], F32, isOutput=True)

    def r(ap):
        return ap.bitcast(F32R) if use_f32r else ap

    from contextlib import ExitStack

    with tile.TileContext(nc) as tc, ExitStack() as ctx:
        consts = ctx.enter_context(tc.tile_pool(name="consts", bufs=1))

        qkv_sb = {}
        vh = []
        qkvpool = ctx.enter_context(tc.tile_pool(name="qkv", bufs=1))

        # ---- Phase 1: const loads + projections (contract over E, 6 blocks) ----
        with (
            tc.tile_pool(name="wstg", bufs=2) as wstg,
            tc.tile_pool(name="xin", bufs=6) as xin_pool,
            tc.tile_pool(name="vt", bufs=1) as vt_pool,
            tc.tile_pool(name="pproj", bufs=4, space="PSUM") as pproj,
            tc.tile_pool(name="ptr", bufs=2, space="PSUM") as ptr,
        ):
            w_sb = {}
            for name, dram in (("q", wq), ("k", wk), ("v", wv)):
                td = wstg.tile([128, EB, C], F32, tag="wd")
                nc.sync.dma_start(
                    out=td, in_=dram[:].rearrange("(eb p) c -> p eb c", p=128)
                )
                t = consts.tile([128, EB, C], F32, tag=f"w_{name}")
                nc.vector.tensor_copy(t[:].bitcast(F32R), td)
                w_sb[name] = t
            wo_sb = []
            for h in range(HPC):
                td = wstg.tile([D, E], F32, tag="wod")
                nc.sync.dma_start(out=td, in_=wo[h * D : (h + 1) * D, :])
                t = consts.tile([D, E], F32, tag=f"wo_{h}")
                nc.vector.tensor_copy(t[:].bitcast(F32R), td)
                wo_sb.append(t)
            b_sb = consts.tile([D, 9], F32)
            nc.sync.dma_start(out=b_sb, in_=bp[:])
            ones_sb = consts.tile([128, 1], F32)
            nc.vector.memset(ones_sb, 1.0)
            ones_row = consts.tile([1, 128], F32)
            nc.vector.memset(ones_row, 1.0)
            identd = wstg.tile([D, D], F32, tag="identd")
            nc.sync.dma_start(out=identd, in_=idin[:])
            ident = consts.tile([D, D], F32)
            nc.vector.tensor_copy(ident, identd)
            # PE observer: one transpose consuming the last DVE funnel copy so
            # later matmuls see all funnel ticks without extra waits
            obs = ptr.tile([D, D], F32, tag="ptr")
            nc.tensor.transpose(obs, ident, ident)

            for ti, (tname, dram) in enumerate((("q", xq), ("k", xk), ("v", xv))):
                xin = []
                for eb in range(EB):
                    xt = xin_pool.tile([128, S], F32, tag="xin")
                    nc.sync.dma_start(
                        out=xt[:].bitcast(F32R),
                        in_=dram[eb * 128 : (eb + 1) * 128, :].bitcast(F32R),
                    )
                    xin.append(xt)
                for h in range(HPC):
                    if tname == "v":
                        qt = vt_pool.tile([D, S], F32, tag=f"vt{h}")
                    else:
                        qt = qkvpool.tile([D, S], F32, tag=f"{tname}{h}")
                    for nch in range(QC):
                        ps = pproj.tile([D, QCH], F32, tag="pproj")
                        for eb in range(EB):
                            nc.tensor.matmul(
                                ps,
                                lhsT=r(w_sb[tname][:, eb, h * D : (h + 1) * D]),
                                rhs=r(xin[eb][:, nch * QCH : (nch + 1) * QCH]),
                                start=(eb == 0),
                                stop=(eb == EB - 1),
                            )
                        nc.vector.tensor_scalar_add(
                            out=qt[:, nch * QCH : (nch + 1) * QCH].bitcast(F32R),
                            in0=ps,
                            scalar1=b_sb[:, ti * HPC + h : ti * HPC + h + 1],
                        )
                    qkv_sb[(tname, h)] = qt
                if tname != "v":
                    # PE observer: absorb this tensor's last bias-add tick
                    obs = ptr.tile([128, D], F32, tag="ptr")
                    nc.tensor.transpose(
                        obs, qkv_sb[(tname, HPC - 1)][:, S - 128 : S], ident
                    )

            # V natural layout per head: [128 (k-block rows), kb, 65] via PE
            # transpose; column D is all-ones so the ctx matmul also produces
            # the softmax normalizer Z as PSUM row D.
            for h in range(HPC):
                vt = qkv_sb[("v", h)]
                vht = qkvpool.tile([128, KB, D + 1], F32, tag=f"vh{h}")
                nc.vector.tensor_copy(
                    vht[:, :, D : D + 1].bitcast(F32R),
                    ones_sb[:, None, 0:1].broadcast_to([128, KB, 1]),
                )
                for kb in range(KB):
                    pst = ptr.tile([128, D], F32, tag="ptr")
                    nc.tensor.transpose(
                        pst, vt[:, kb * 128 : (kb + 1) * 128], ident
                    )
                    nc.vector.tensor_copy(vht[:, kb, 0:D].bitcast(F32R), pst)
                vh.append(vht)

        # ---- Phase 2: attention + output projection ----
        with (
            tc.tile_pool(name="psc", bufs=2, space="PSUM") as psc,
            tc.tile_pool(name="psctx", bufs=1, space="PSUM") as psctx,
            tc.tile_pool(name="psrb", bufs=1, space="PSUM") as psrb,
            tc.tile_pool(name="pso", bufs=1, space="PSUM") as pso_pool,
            tc.tile_pool(name="expp", bufs=2) as expp,
            tc.tile_pool(name="rp", bufs=2) as rp,
            tc.tile_pool(name="rbp", bufs=2) as rbp,
            tc.tile_pool(name="ctxup", bufs=2) as ctxup,
            tc.tile_pool(name="ctxtp", bufs=6) as ctxtp,
            tc.tile_pool(name="outp", bufs=2) as outp,
            tc.tile_pool(name="stgp", bufs=2) as stgp,
        ):
            for qc in range(QC):
                ctxt_q = []
                for h in range(HPC):
                    qt = qkv_sb[("q", h)]
                    kt = qkv_sb[("k", h)]
                    exp_chunk = expp.tile([128, KB, QCH], F32, tag="exp")
                    # scores + exp, two k-blocks per PSUM tile / ACT op
                    for kp in range(KB // 2):
                        ps = psc.tile([128, 2, QCH], F32, tag="psc")
                        for j in range(2):
                            nc.tensor.matmul(
                                ps[:, j, :],
                                lhsT=r(kt[:, (2 * kp + j) * 128 : (2 * kp + j + 1) * 128]),
                                rhs=r(qt[:, qc * QCH : (qc + 1) * QCH]),
                                start=True,
                                stop=True,
                                skip_group_check=True,
                            )
                        nc.scalar.activation(
                            out=exp_chunk[:, 2 * kp : 2 * kp + 2, :].bitcast(F32R),
                            in_=ps,
                            func=ActF.Exp,
                            scale=0.125,
                        )
                    # ctx accumulation; ones column makes row D the Z row
                    cps = psctx.tile([D + 1, QCH], F32, tag="cps")
                    for kb in range(KB):
                        nc.tensor.matmul(
                            cps,
                            lhsT=r(vh[h][:, kb, :]),
                            rhs=r(exp_chunk[:, kb, :]),
                            start=(kb == 0),
                            stop=(kb == KB - 1),
                        )
                    # Z row -> SBUF (1 lane), broadcast via K=1 ones matmul,
                    # reciprocal on all 128 lanes
                    zrow = rp.tile([1, QCH], F32, tag="zrow")
                    nc.scalar.activation(
                        out=zrow[:].bitcast(F32R), in_=cps[D : D + 1, :], func=ActF.Copy
                    )
                    rb_ps = psrb.tile([128, QCH], F32, tag="rbps")
                    nc.tensor.matmul(
                        rb_ps, lhsT=r(ones_row), rhs=r(zrow), start=True, stop=True
                    )
                    recipB = rbp.tile([128, QCH], F32, tag="recipB")
                    nc.vector.reciprocal(recipB, rb_ps)

                    ctxu = ctxup.tile([D, QCH], F32, tag="ctxu")
                    nc.scalar.activation(out=ctxu, in_=cps[0:D, :], func=ActF.Copy)
                    ctxt = ctxtp.tile([D, QCH], F32, tag="ctxt")
                    nc.vector.tensor_mul(ctxt[:].bitcast(F32R), ctxu, recipB[0:D, :])
                    ctxt_q.append(ctxt)

                    # normalize attn on GpSimd through staging tiles (keeps
                    # DVE free; attn output stays full fp32)
                    for g in range(4):
                        stg = stgp.tile([128, 4, QCH], F32, tag="stg")
                        nc.gpsimd.tensor_mul(
                            stg,
                            exp_chunk[:, g * 4 : (g + 1) * 4, :],
                            recipB[:, None, :].broadcast_to([128, 4, QCH]),
                        )
                        nc.sync.dma_start(
                            out=attn_o[h, qc, :, g * 4 : (g + 1) * 4, :], in_=stg
                        )

                # output projection for this q-chunk (s == q rows)
                for sb in range(4):
                    poa = pso_pool.tile([128, 512], F32, tag="poa")
                    pob = pso_pool.tile([128, 256], F32, tag="pob")
                    for h in range(HPC):
                        lh = r(ctxt_q[h][:, sb * 128 : (sb + 1) * 128])
                        nc.tensor.matmul(
                            poa, lhsT=lh, rhs=r(wo_sb[h][:, 0:512]),
                            start=(h == 0), stop=(h == HPC - 1),
                        )
                        nc.tensor.matmul(
                            pob, lhsT=lh, rhs=r(wo_sb[h][:, 512:768]),
                            start=(h == 0), stop=(h == HPC - 1),
                        )
                    ot = outp.tile([128, E], F32, tag="out")
                    nc.vector.tensor_copy(ot[:, 0:512], poa)
                    nc.vector.tensor_copy(ot[:, 512:768], pob)
                    nc.sync.dma_start(out=out_o[qc * 4 + sb], in_=ot)
    nc.compile()
    return nc


def _get_nc():
    use_f32r = os.environ.get("BASS_MM_DTYPE", "f32r") == "f32r"
    key = ("nc", use_f32r)
    if key not in _CACHE:
        _CACHE[key] = _build(use_f32r)
    return _CACHE[key]


def make_in_maps(query, key, value, W_Q, W_K, W_V, b_Q, b_K, b_V, W_O, b_O):
    query = np.asarray(query, np.float32)
    key = np.asarray(key, np.float32)
    value = np.asarray(value, np.float32)
    in_maps = []
    for c in range(NCORES):
        b, hs = c // 4, c % 4
        ch0 = C * hs
        bpk = np.zeros((D, 9), np.float32)
        for ti, bb in enumerate((b_Q, b_K, b_V)):
            bb = np.asarray(bb, np.float32)
            for h in range(HPC):
                bpk[:, ti * HPC + h] = bb[ch0 + h * D : ch0 + (h + 1) * D]
        in_maps.append(
            {
                "xq_t": np.ascontiguousarray(query[:, b, :].T),
                "xk_t": np.ascontiguousarray(key[:, b, :].T),
                "xv_t": np.ascontiguousarray(value[:, b, :].T),
                "wq_t": np.ascontiguousarray(np.asarray(W_Q)[ch0 : ch0 + C, :].T),
                "wk_t": np.ascontiguousarray(np.asarray(W_K)[ch0 : ch0 + C, :].T),
                "wv_t": np.ascontiguousarray(np.asarray(W_V)[ch0 : ch0 + C, :].T),
                "wo_t": np.ascontiguousarray(np.asarray(W_O)[:, ch0 : ch0 + C].T),
                "b_pack": bpk,
                "ident64": np.eye(D, dtype=np.float32),
            }
        )
    return in_maps


def assemble(results, b_O):
    out = np.zeros((S, B, E), np.float32)
    attn = np.empty((B * H, S, S), np.float32)
    for c in range(NCORES):
        b, hs = c // 4, c % 4
        out[:, b, :] += results[c]["out_partial"].reshape(S, E)
        a = results[c]["attn_t"]  # [HPC, QC, 128, KB, QCH]
        n0 = H * b + HPC * hs
        attn[n0 : n0 + HPC] = a.transpose(0, 1, 4, 3, 2).reshape(HPC, S, S)
    out += np.asarray(b_O, np.float32)[None, None, :]
    return out, attn


_LAST = {}


def _ensure_ntff_hook():
    """bass_utils' trace path imports antenv.axon_hooks, which this image
    lacks. Recreate it: a tiny registry returning the ctypes NTFF hook
    from trn_agent_boot (dlopens libaxon_pjrt.so directly)."""
    import types

    if "antenv.axon_hooks" in sys.modules:
        return
    try:
        from trn_agent_boot.trn_boot import _ntff_profile_via_ctypes

        hook = _ntff_profile_via_ctypes("/opt/axon/libaxon_pjrt.so")
    except Exception:
        hook = None
    mod = types.ModuleType("antenv.axon_hooks")
    mod._hook = hook
    mod.get_axon_ntff_profile_hook = lambda: mod._hook
    mod.set_axon_ntff_profile_hook = lambda h: setattr(mod, "_hook", h)
    sys.modules["antenv.axon_hooks"] = mod


def kernel(query, key, value, W_Q, W_K, W_V, b_Q, b_K, b_V, W_O, b_O):
    nc = _get_nc()
    in_maps = make_in_maps(
        query, key, value, W_Q, W_K, W_V, b_Q, b_K, b_V, W_O, b_O
    )
    trace = os.environ.get("BASS_KERNEL_TRACE", "0") == "1"
    if trace:
        _ensure_ntff_hook()
    res = run_bass_kernel_spmd(nc, in_maps, core_ids=list(range(NCORES)), trace=trace)
    _LAST["exec_time_ns"] = res.exec_time_ns
    _LAST["profile_json"] = res.profile_json
    if res.exec_time_ns is not None:
        print(f"HW exec time: {res.exec_time_ns} ns")
    return assemble(res.results, b_O)
